# revision 9
# baseline (speedup 1.0000x reference)
"""Causal multi-head attention on 8 Trainium2 NeuronCores — v2.

Problem: x[2,2048,1024] @ W_Q/K/V[1024,1024] -> 16-head causal attention
(d_head=64) -> @ W_O[1024,1024].

Sharding: DP(batch=2) x TP(head-groups=4). Core i owns batch i//4 and
heads 4k..4k+3 where k = i%4 (columns [256k:256k+256) of W_Q/K/V, rows
[256k:256k+256) of W_O). Each core emits a partial [1024, 2048] output
for its batch; host sums groups of 4 and transposes.

All matmul inputs bf16 (f32 PSUM accumulate). Design notes:
  - Attention runs in two "waves" (head pairs), so PSUM fits: scores
    double-buffered [128,2,512] (4 banks) + PV accum [65,2,512]
    (2 banks) + utility [128,512] x2 (2 banks) = 8 banks.
  - exp batched over the head pair: one ACTIVATE per (jj, chunk) covers
    [128, 2, width] -> halves ScalarE fixed overhead.
  - V projected directly in [token, dim] orientation (x-chunk stationary,
    W_V moving) — no PE transposes.
  - Projection work is emitted just-in-time into wave-A chunk loops (Q
    before its q-tile, K/V before their k-chunks); W_O blocks interleave
    into wave-B (jj order 1,2,3,0 so the tail lands on the shortest jj).
    Keeps the PE stream dense so the HAM clock gate stays at 2.4 GHz.
  - W_O results bounce PSUM->SBUF as bf16 (ScalarE/DVE alternating) and
    store with one DMA per token tile; host sums bf16 partials.
  - Softmax denominator via ones-column in vn ([65]-wide PV stationary).
    PV accumulators drain to SBUF incrementally (column quarters finalize
    as the causal diagonal passes), so the PSUM bank handoff to the next
    q-tile never stalls. Normalize = reciprocal_approx_fast on the
    denominator row, gpsimd partition broadcast, DVE multiply — all off
    the PE critical path; the final q-tile normalizes straight from PSUM.
"""

import numpy as np

import concourse.bass as bass
import concourse.tile as tile
from concourse import bacc, mybir
from concourse.bass_utils import run_bass_kernel_spmd

F32 = mybir.dt.float32
BF16 = mybir.dt.bfloat16

N_CORES = 8
P = 128
D = 1024          # d_model
B = 2             # batch
S = 2048          # seq len (= tokens per core)
TT = 512          # token tile
NT = S // TT      # 4 token tiles
KD = D // P       # 8 contraction chunks for projections
CB = S // P       # 16 k-chunks
HL = 4            # heads per core
DL = 256          # dims per core (2 ptiles of 128)
DH = 64           # head dim
EXP = mybir.ActivationFunctionType.Exp


def _body(tc):
    nc = tc.nc
    # all host-side pre-arranged so every DMA is contiguous
    xT = nc.dram_tensor("xT", [P, KD, S], BF16, kind="ExternalInput").ap()
    wq = nc.dram_tensor("wq", [P, KD, DL], BF16, kind="ExternalInput").ap()
    wk = nc.dram_tensor("wk", [P, KD, DL], BF16, kind="ExternalInput").ap()
    wv = nc.dram_tensor("wv", [P, KD, DL], BF16, kind="ExternalInput").ap()
    wo = nc.dram_tensor("wo", [P, 2, D], BF16, kind="ExternalInput").ap()
    outT = nc.dram_tensor("outT", [P, KD, S], BF16, kind="ExternalOutput").ap()

    import contextlib
    with contextlib.ExitStack() as ctx:
        const = ctx.enter_context(tc.tile_pool(name="const", bufs=1))
        wpool = ctx.enter_context(tc.tile_pool(name="wpool", bufs=1))
        xpool = ctx.enter_context(tc.tile_pool(name="xpool", bufs=2))
        persist = ctx.enter_context(tc.tile_pool(name="persist", bufs=1))
        probs_p = ctx.enter_context(tc.tile_pool(name="probs", bufs=4))
        stage = ctx.enter_context(tc.tile_pool(name="stage", bufs=2))
        psum = ctx.enter_context(tc.tile_pool(name="psum", bufs=1, space="PSUM"))

        # --- constants -----------------------------------------------------
        # mask_band[k, q] = 1.0 if q >= k else 0.0
        mask_band = const.tile([P, P], BF16)
        nc.any.memset(mask_band[:], 1.0)
        nc.gpsimd.affine_select(
            out=mask_band[:],
            in_=mask_band[:],
            compare_op=mybir.AluOpType.is_ge,
            fill=0.0,
            base=0,
            pattern=[[1, P]],
            channel_multiplier=-1,
        )
        # preload the exp table set while projections run
        scr = const.tile([1, 1], F32)
        nc.any.memset(scr[:], 0.0)
        nc.scalar.activation(scr[:], scr[:], EXP)

        # --- weights -------------------------------------------------------
        wq_sb = wpool.tile([P, KD, DL], BF16)
        wk_sb = wpool.tile([P, KD, DL], BF16)
        wv_sb = wpool.tile([P, KD, DL], BF16)
        wo_sb = wpool.tile([P, 2, D], BF16)

        # --- persistent activations ---------------------------------------
        qT = persist.tile([P, 2, S], BF16)    # [dim%128, ptile, token]
        kT = persist.tile([P, 2, S], BF16)
        attnT = persist.tile([P, 2, S], BF16)
        vn = persist.tile([P, CB, HL, DH + 1], BF16)  # [tok, chunk, head, d|1]
        nc.any.memset(vn[:, :, :, DH], 1.0)
        pvsb = persist.tile([DH + 1, NT, 2, TT], F32)  # [d|1, jj, hh, q]


        # --- projection work for one token tile, as emission groups -------
        # Returns (load, [Q chains], [K chains], [V chunks]). Q must be
        # emitted before wave-A jj=t; K/V only before chunk 4t of jj=t.
        xts = {t: xpool.tile([P, KD, TT], BF16, name=f"xt_{t}")
               for t in range(NT)}

        def proj_load(t):
            def run():
                # per-chunk DMAs: subtile deps let chain c start as soon as
                # its own chunk lands, instead of waiting for the full tile
                for c in range(KD):
                    nc.sync.dma_start(xts[t][:, c, :],
                                      xT[:, c, bass.ts(t, TT)])
            return run

        def proj_tile_groups(t):
            tsl = bass.ts(t, TT)
            xt = xts[t]

            def qk_chain(wsb, dst, pt):
                def run():
                    ps = psum.tile([P, TT], F32, tag="u", bufs=2,
                                   name=f"ps_{t}_{pt}")
                    for c in range(KD):
                        nc.tensor.matmul(ps[:], wsb[:, c, bass.ts(pt, P)],
                                         xt[:, c, :],
                                         start=(c == 0), stop=(c == KD - 1))
                    nc.vector.tensor_copy(dst[:, pt, tsl], ps[:])
                return run

            # V directly in [token, dim] orientation: x chunk stationary.
            def v_chunk(s_):
                def run():
                    ch = t * 4 + s_
                    ps = psum.tile([P, DL], F32, tag="u", bufs=2,
                                   name=f"psv_{t}_{s_}")
                    for c in range(KD):
                        nc.tensor.matmul(ps[:], xt[:, c, bass.ts(s_, P)],
                                         wv_sb[:, c, :],
                                         start=(c == 0), stop=(c == KD - 1))
                    # [128 tok, 4*64 dims] -> vn[:, ch, h, 0:64]
                    nc.vector.tensor_copy(
                        vn[:, ch, :, 0:DH],
                        ps.rearrange("p (h d) -> p h d", h=HL))
                return run

            qs = [qk_chain(wq_sb, qT, pt) for pt in range(2)]
            ks = [qk_chain(wk_sb, kT, pt) for pt in range(2)]
            vs = [v_chunk(s_) for s_ in range(4)]
            return qs, ks, vs

        # --- W_O for one token tile, as one emission group ----------------
        def wo_group(jx):
            jsl = bass.ts(jx, TT)

            def run():
                ob = stage.tile([P, KD, TT], BF16, tag="ob", bufs=2,
                                name=f"ob_{jx}")
                for f in range(KD):
                    wu = psum.tile([P, TT], F32, tag="u", bufs=2,
                                   name=f"wu_{jx}_{f}")
                    nc.tensor.matmul(wu[:], wo_sb[:, 0, bass.ts(f, P)],
                                     attnT[:, 0, jsl], start=True, stop=False)
                    nc.tensor.matmul(wu[:], wo_sb[:, 1, bass.ts(f, P)],
                                     attnT[:, 1, jsl], start=False, stop=True)
                    if f % 2 == 0:
                        nc.scalar.copy(ob[:, f, :], wu[:])
                    else:
                        nc.vector.tensor_copy(ob[:, f, :], wu[:])
                    nc.sync.dma_start(outT[:, f, jsl], ob[:, f, :])
            return run

        # --- attention wave: one head pair (ptile), all q-tiles -----------
        def norm_jj(pt, jj):
            jsl = bass.ts(jj, TT)
            for hh in range(2):
                # reciprocal_approx_fast misreads inputs at base partition
                # 64 (custom-DVE uop quirk) — bounce the row to partition 0.
                dcp = stage.tile([1, TT], F32, tag="dcp",
                                 name=f"dcp_{pt}_{jj}_{hh}")
                nc.vector.tensor_copy(dcp[:], pvsb[DH:DH + 1, jj, hh, :])
                rc = stage.tile([1, TT], F32, tag="rc",
                                name=f"rc_{pt}_{jj}_{hh}")
                nc.vector.reciprocal_approx_fast(out=rc[:], in_=dcp[:])
                rb = stage.tile([DH, TT], F32, tag="rb",
                                name=f"rb_{pt}_{jj}_{hh}")
                nc.gpsimd.partition_broadcast(rb[:], rc[:])
                nc.vector.tensor_mul(
                    attnT[DH * hh:DH * hh + DH, pt, jsl],
                    pvsb[0:DH, jj, hh, :], rb[:])

        def wave(pt, extra_per_jj, order=None, fast_last=False):
            """extra_per_jj[i]: emission groups interleaved into the i-th
            processed jj's chunk loop (all emitted before its normalize)."""
            order = list(order) if order else list(range(NT))
            for idx, jj in enumerate(order):
                extra = list(extra_per_jj[idx])
                ncb = 4 * (jj + 1)
                jsl = bass.ts(jj, TT)
                pv = psum.tile([DH + 1, 2, TT], F32, tag="pv", bufs=1,
                               name=f"pv_{pt}_{jj}")

                def pv_step(cb, pr, jj=jj, ncb=ncb, pv=pv):
                    r = cb - 4 * jj
                    lo = P * r if r > 0 else 0
                    for hh in range(2):
                        nc.tensor.matmul(pv[:, hh, lo:],
                                         vn[:, cb, 2 * pt + hh, :],
                                         pr[:, hh, lo:],
                                         start=(cb == 0), stop=(cb == ncb - 1))

                pending = None
                for cb in range(ncb):
                    r = cb - 4 * jj
                    lo = P * r if r > 0 else 0
                    csl = bass.ts(cb, P)
                    sps = psum.tile([P, 2, TT], F32, tag="s", bufs=2,
                                    name=f"sps_{pt}_{jj}_{cb}")
                    for hh in range(2):
                        hp = slice(DH * hh, DH * hh + DH)
                        nc.tensor.matmul(sps[:, hh, lo:], kT[hp, pt, csl],
                                         qT[hp, pt, jsl][:, lo:],
                                         start=True, stop=True)
                    pr = probs_p.tile([P, 2, TT], BF16, tag="pr",
                                      name=f"pr_{pt}_{jj}_{cb}")
                    nc.scalar.activation(pr[:, :, lo:], sps[:, :, lo:],
                                         EXP, scale=0.125)
                    if r >= 0:
                        for hh in range(2):
                            nc.vector.tensor_mul(pr[:, hh, bass.ts(r, P)],
                                                 pr[:, hh, bass.ts(r, P)],
                                                 mask_band[:])
                    if pending is not None:
                        pv_step(cb - 1, pending)
                        rq = cb - 1 - 4 * jj
                        if rq >= 0 and not (fast_last and idx == NT - 1):
                            # columns [128rq,128rq+128) of pv are final now:
                            # drain incrementally so the bank frees right
                            # after the last PV instead of one big copy late
                            qsl = bass.ts(rq, P)
                            if pt == 0:
                                nc.scalar.copy(pvsb[:, jj, :, qsl],
                                               pv[:, :, qsl])
                            else:
                                nc.vector.tensor_copy(pvsb[:, jj, :, qsl],
                                                      pv[:, :, qsl])
                    pending = pr
                    # spread the extra groups across remaining chunk slots
                    k = -(-len(extra) // (ncb - cb)) if extra else 0
                    for _ in range(k):
                        extra.pop(0)()
                pv_step(ncb - 1, pending)
                if not (fast_last and idx == NT - 1):
                    # final quarter always on ScalarE: it frees the pv bank
                    # that gates the next jj's first PV, and must not queue
                    # behind pending DVE work
                    qsl = bass.ts(3, P)
                    nc.scalar.copy(pvsb[:, jj, :, qsl], pv[:, :, qsl])
                while extra:
                    extra.pop(0)()

                if fast_last and idx == NT - 1:
                    # final jj of the kernel: nothing needs the PSUM banks
                    # next, so normalize straight out of PSUM (shortest
                    # latency into the last W_O block)
                    dcp2 = stage.tile([1, 2, TT], F32, tag="dcp2",
                                      name=f"dcp2_{pt}_{jj}")
                    nc.vector.tensor_copy(dcp2[:], pv[DH:DH + 1, :, :])
                    rc2 = stage.tile([1, 2, TT], F32, tag="rc2",
                                     name=f"rc2_{pt}_{jj}")
                    nc.vector.reciprocal_approx_fast(out=rc2[:], in_=dcp2[:])
                    for hh in range(2):
                        rb = stage.tile([DH, TT], F32, tag="rb",
                                        name=f"rbl_{pt}_{jj}_{hh}")
                        nc.gpsimd.partition_broadcast(rb[:], rc2[:, hh, :])
                        nc.vector.tensor_mul(
                            attnT[DH * hh:DH * hh + DH, pt, jsl],
                            pv[0:DH, hh, :], rb[:])
                    continue
                norm_jj(pt, jj)

        # --- schedule ------------------------------------------------------
        # DMA issue order = packet priority on the shared queues: the first
        # Q chain needs wq + xt0 only; wo isn't read until wave B.
        nc.sync.dma_start(wq_sb[:], wq)
        proj_load(0)()
        nc.sync.dma_start(wk_sb[:], wk)
        nc.sync.dma_start(wv_sb[:], wv)
        proj_load(1)()
        nc.sync.dma_start(wo_sb[:], wo)
        pg = {t: proj_tile_groups(t) for t in range(NT)}
        for g in pg[0][0] + pg[0][1] + pg[0][2]:   # tile 0: Q, K, V
            g()
        wave(0, [
            pg[1][0] + [proj_load(2)],                      # jj0: Q1, L2
            pg[1][1] + pg[1][2] + pg[2][0] + [proj_load(3)],  # K1 V1 Q2 L3
            pg[2][1] + pg[2][2] + pg[3][0],                 # K2 V2 Q3
            pg[3][1] + pg[3][2],                            # K3 V3
        ])
        wave(1, [[], [wo_group(1)], [wo_group(2)], [wo_group(3)]],
             order=[1, 2, 3, 0], fast_last=True)
        wo_group(0)()


_NC_CACHE = None


def _get_nc():
    global _NC_CACHE
    if _NC_CACHE is None:
        nc = bacc.Bacc("TRN2", target_bir_lowering=False, debug=False,
                       num_devices=N_CORES)
        with tile.TileContext(nc) as tc:
            _body(tc)
        nc.compile()
        _NC_CACHE = nc
    return _NC_CACHE


def _pom(w):
    """[o*P+p, m] -> [p, o, m] (contiguous)."""
    o = w.shape[0] // P
    return np.ascontiguousarray(w.reshape(o, P, -1).transpose(1, 0, 2))


def _in_maps(x, W_Q, W_K, W_V, W_O):
    from ml_dtypes import bfloat16
    x = np.asarray(x, dtype=np.float32)
    W_Q = np.asarray(W_Q, dtype=np.float32).astype(bfloat16)
    W_K = np.asarray(W_K, dtype=np.float32).astype(bfloat16)
    W_V = np.asarray(W_V, dtype=np.float32).astype(bfloat16)
    W_O = np.asarray(W_O, dtype=np.float32).astype(bfloat16)
    xTs = [_pom(np.ascontiguousarray(x[b].T).astype(bfloat16))
           for b in range(B)]
    maps = []
    for i in range(N_CORES):
        b, k = i // 4, i % 4
        sl = slice(DL * k, DL * k + DL)
        maps.append({
            "xT": xTs[b],
            "wq": _pom(W_Q[:, sl]),
            "wk": _pom(W_K[:, sl]),
            "wv": _pom(W_V[:, sl]),
            "wo": _pom(W_O[sl, :]),
        })
    return maps


def _gather(results):
    out = np.zeros([B, S, D], np.float32)
    for b in range(B):
        acc = np.zeros([D, S], np.float64)
        for i in range(4 * b, 4 * b + 4):
            # [p, o, n] -> [o*P+p, n]
            acc += np.asarray(results[i]["outT"],
                              np.float32).transpose(1, 0, 2).reshape(D, S)
        out[b] = acc.T
    return out


def kernel(x, W_Q, W_K, W_V, W_O):
    nc = _get_nc()
    res = run_bass_kernel_spmd(nc, _in_maps(x, W_Q, W_K, W_V, W_O),
                               core_ids=list(range(N_CORES)))
    return _gather(res.results)


LAST_RESULT = None


def kernel_profiled(x, W_Q, W_K, W_V, W_O):
    """Like kernel() but with NTFF tracing; returns (output, exec_time_ns)."""
    import os
    global LAST_RESULT
    nc = _get_nc()
    res = run_bass_kernel_spmd(nc, _in_maps(x, W_Q, W_K, W_V, W_O),
                               core_ids=list(range(N_CORES)), trace=True,
                               tmpdir=os.environ.get("BASS_TRACE_DIR"))
    LAST_RESULT = res
    return _gather(res.results), res.exec_time_ns
